# revision 1
# baseline (speedup 1.0000x reference)
"""Trainium2 Bass kernel for nn_CountingDiceLoss.

Key insight: in the reference, the cross-entropy term uses log_softmax over a
single-channel axis (identically zero) and a target clipped to index 0, so the
CE contribution is exactly 0 and the entire density-map computation (cent_i,
cent_j, bbox) is dead code.  The output reduces to the soft-dice loss over
classes 1 and 2:

    dc[b,c]  = (2*tp + s) / (sp + cnt + s),   s = 1e-5
    tp[b,c]  = sum_px softmax(x[b,:3])[c] * (y[b]==c)
    sp[b,c]  = sum_px softmax(x[b,:3])[c]
    cnt[b,c] = sum_px (y[b]==c)
    loss     = -mean_{b, c in {1,2}} dc[b,c]

Sharding: data-parallel over batch B=8, one sample per NeuronCore.  Each core
streams its sample's 3 class channels of x (12MB) + y (4MB), computes the
softmax in fp16 (exp on ACT, r = exp(-ln(den)) on ACT since the Reciprocal
activation is banned), masks/products on DVE, and reduces with TensorEngine
matmuls against a ones-vector into PSUM.  Output per core: 6 partial sums.
Host combines in float64.
"""

import os
import sys

import numpy as np

for _p in ("/opt/trn_rl_repo",):
    if _p not in sys.path and os.path.isdir(_p):
        sys.path.append(_p)

from contextlib import ExitStack

import concourse.bass as bass
import concourse.tile as tile
from concourse import bacc, mybir
from concourse.bass_utils import run_bass_kernel_spmd

P = 128          # SBUF partitions
WTOT = 8192      # free-dim length of one 1024x1024 plane laid out as [128, 8192]
FREE = int(os.environ.get("K_FREE", "2048"))  # chunk free size
NCH = WTOT // FREE
MM = 512         # matmul free size (one PSUM bank of fp32)
NQ = 6           # reduced quantities: sp1, sp2, tp1, tp2, cnt1, cnt2
NBUF = int(os.environ.get("K_BUFS", "2"))
SMOOTH = 1e-5

f16 = mybir.dt.float16
f32 = mybir.dt.float32
i32 = mybir.dt.int32
AF = mybir.ActivationFunctionType
ALU = mybir.AluOpType


def _emit(ctx: ExitStack, tc: "tile.TileContext", out_ap, x_ap, y_ap, repeat=1,
          variant="full"):
    nc = tc.nc

    xin = ctx.enter_context(tc.tile_pool(name="xin", bufs=NBUF))
    yin = ctx.enter_context(tc.tile_pool(name="yin", bufs=NBUF))
    work = ctx.enter_context(tc.tile_pool(name="work", bufs=NBUF))
    red = ctx.enter_context(tc.tile_pool(name="red", bufs=NBUF))
    singles = ctx.enter_context(tc.tile_pool(name="singles", bufs=1))
    psum = ctx.enter_context(tc.tile_pool(name="psum", bufs=1, space="PSUM"))

    # column-selector stationary matrices: colsel[j] is [128, NQ] with ones in
    # column j.  matmul(acc, colsel[j], rhs) adds rhs's partition-sum into PSUM
    # row j and +0 into the other rows, so all six quantities share one bank.
    colsel = []
    for j in range(NQ):
        cs = singles.tile([P, NQ], f16, tag=f"colsel{j}")
        nc.vector.memset(cs, 0.0)
        nc.vector.memset(cs[:, j : j + 1], 1.0)
        colsel.append(cs)

    # one PSUM bank; row j accumulates quantity j as [1, MM] partials
    acc = psum.tile([NQ, MM], f32)

    # chunk plan: (offset, size) pairs covering WTOT columns.  "tailsplit"
    # shrinks the final chunks so the post-last-DMA dependent-compute tail
    # is short; plain plan is uniform FREE-sized chunks.
    if variant == "tailsplit":
        plan = [(o, FREE) for o in range(0, WTOT - FREE, FREE)]
        o = WTOT - FREE
        plan += [(o, 1024), (o + 1024, 512), (o + 1536, 256), (o + 1792, 256)]
    else:
        plan = [(o, FREE) for o in range(0, WTOT, FREE)]

    # repeat>1 replays the whole body for slope-based device timing; the
    # extra passes accumulate into the same PSUM rows (results unused then)
    for rep, (k, (off, csz)) in (
        (r, c) for r in range(repeat) for c in enumerate(plan)
    ):
        first_it = rep == 0 and k == 0
        last_it = rep == repeat - 1 and k == len(plan) - 1
        sl = slice(off, off + csz)

        x0_t = xin.tile([P, FREE], f32, tag="x0")

        x0 = x0_t[:, :csz]
        x1_t = xin.tile([P, FREE], f32, tag="x1")
        x1 = x1_t[:, :csz]
        x2_t = xin.tile([P, FREE], f32, tag="x2")
        x2 = x2_t[:, :csz]
        yt_t = yin.tile([P, FREE], i32, tag="yt")
        yt = yt_t[:, :csz]
        nc.sync.dma_start(out=x0, in_=x_ap[0, :, sl])
        nc.sync.dma_start(out=x1, in_=x_ap[1, :, sl])
        nc.sync.dma_start(out=x2, in_=x_ap[2, :, sl])
        nc.sync.dma_start(out=yt, in_=y_ap[:, sl])
        if variant == "dmaonly":
            # consume one column of each tile so DCE can't drop the loads
            junk = work.tile([P, 4], f32, tag="junk")
            nc.vector.tensor_scalar(junk[:, 0:1], x0[:, 0:1], 0.0, None, ALU.add)
            nc.vector.tensor_scalar(junk[:, 1:2], x1[:, 0:1], 0.0, None, ALU.add)
            nc.vector.tensor_scalar(junk[:, 2:3], x2[:, 0:1], 0.0, None, ALU.add)
            nc.vector.tensor_scalar(junk[:, 3:4], yt[:, 0:1], 0.0, None, ALU.add)
            continue

        e0_t = work.tile([P, FREE], f16, tag="e0")

        e0 = e0_t[:, :csz]
        e1_t = work.tile([P, FREE], f16, tag="e1")
        e1 = e1_t[:, :csz]
        e2_t = work.tile([P, FREE], f16, tag="e2")
        e2 = e2_t[:, :csz]
        nc.scalar.activation(e0, x0, AF.Exp)
        nc.scalar.activation(e1, x1, AF.Exp)
        nc.scalar.activation(e2, x2, AF.Exp)

        d01_t = work.tile([P, FREE], f16, tag="d01")

        d01 = d01_t[:, :csz]
        den_t = work.tile([P, FREE], f16, tag="den")
        den = den_t[:, :csz]
        nc.vector.tensor_add(d01, e0, e1)
        nc.vector.tensor_add(den, d01, e2)

        # softmax denominator reciprocal as exp(-ln(den)): Ln and Exp share an
        # ACT table set; the Reciprocal activation is banned for accuracy.
        rr_t = work.tile([P, FREE], f16, tag="rr")
        rr = rr_t[:, :csz]
        if variant == "norecip":  # timing probe only — wrong values
            nc.vector.tensor_copy(rr, den)
        elif variant == "dverecip" or (
            variant in ("hybrid", "hybrid2") and k >= len(plan) - (
                1 if variant == "hybrid" else 2)
        ):
            with nc.allow_low_precision(reason="fp16 softmax reciprocal"):
                nc.vector.reciprocal(rr, den)
        else:
            lg_t = work.tile([P, FREE], f32, tag="lg")
            lg = lg_t[:, :csz]
            nc.scalar.activation(lg, den, AF.Ln)
            nc.scalar.activation(rr, lg, AF.Exp, scale=-1.0)

        p1_t = red.tile([P, FREE], f16, tag="p1")

        p1 = p1_t[:, :csz]
        p2_t = red.tile([P, FREE], f16, tag="p2")
        p2 = p2_t[:, :csz]
        nc.vector.tensor_mul(p1, e1, rr)
        nc.vector.tensor_mul(p2, e2, rr)

        m1_t = red.tile([P, FREE], f16, tag="m1")

        m1 = m1_t[:, :csz]
        m2_t = red.tile([P, FREE], f16, tag="m2")
        m2 = m2_t[:, :csz]
        nc.vector.tensor_scalar(m1, yt, 1, None, ALU.is_equal)
        nc.vector.tensor_scalar(m2, yt, 2, None, ALU.is_equal)

        q1_t = red.tile([P, FREE], f16, tag="q1")

        q1 = q1_t[:, :csz]
        q2_t = red.tile([P, FREE], f16, tag="q2")
        q2 = q2_t[:, :csz]
        nc.vector.tensor_mul(q1, p1, m1)
        nc.vector.tensor_mul(q2, p2, m2)

        for j, t in enumerate([p1, p2, q1, q2, m1, m2]):
            for s in range(0, csz, MM):
                n = min(MM, csz - s)
                nc.tensor.matmul(
                    acc[:, :n],
                    colsel[j],
                    t[:, s : s + n],
                    start=(first_it and j == 0 and s == 0),
                    stop=(last_it and j == NQ - 1 and s + n == csz),
                )

    res = singles.tile([NQ, 1], f32)
    if variant == "dmaonly":
        nc.vector.memset(res, 0.0)
    else:
        nc.vector.reduce_sum(res, acc, axis=mybir.AxisListType.X)
    nc.sync.dma_start(out=out_ap, in_=res)


_NC_CACHE = {}


def _build_nc(repeat=1, variant="full"):
    key = (repeat, variant)
    if key not in _NC_CACHE:
        nc = bacc.Bacc(
            "TRN2",
            target_bir_lowering=False,
            debug=False,
            num_devices=8,
        )
        x_ap = nc.dram_tensor("xc", [3, P, WTOT], f32, kind="ExternalInput").ap()
        y_ap = nc.dram_tensor("yc", [P, WTOT], i32, kind="ExternalInput").ap()
        out_ap = nc.dram_tensor("out", [NQ, 1], f32, kind="ExternalOutput").ap()
        with tile.TileContext(nc) as tc:
            with ExitStack() as ctx:
                _emit(ctx, tc, out_ap, x_ap, y_ap, repeat=repeat, variant=variant)
        nc.compile()
        _NC_CACHE[key] = nc
    return _NC_CACHE[key]


def _get_nc():
    return _build_nc(1, os.environ.get("K_VARIANT", "full"))


def _run_cores(x: np.ndarray, y: np.ndarray, **spmd_kwargs):
    assert x.shape == (8, 4, 1024, 1024), x.shape
    assert y.shape == (8, 1, 1024, 1024), y.shape
    nc = _get_nc()
    in_maps = []
    for b in range(8):
        xb = np.ascontiguousarray(x[b, :3], dtype=np.float32).reshape(3, P, WTOT)
        yb = np.ascontiguousarray(y[b, 0], dtype=np.int32).reshape(P, WTOT)
        in_maps.append({"xc": xb, "yc": yb})
    return run_bass_kernel_spmd(nc, in_maps, list(range(8)), **spmd_kwargs)


def _combine(results) -> np.float32:
    total = 0.0
    for b in range(8):
        o = np.asarray(results[b]["out"], dtype=np.float64).reshape(NQ)
        sp1, sp2, tp1, tp2, c1, c2 = o
        total += (2.0 * tp1 + SMOOTH) / (sp1 + c1 + SMOOTH)
        total += (2.0 * tp2 + SMOOTH) / (sp2 + c2 + SMOOTH)
    return np.float32(-total / 16.0)


def kernel(x, y, cent_i=None, cent_j=None, bbox=None) -> np.ndarray:
    # cent_i / cent_j / bbox only feed the density map, which is dead code in
    # the reference loss (CE term is identically zero).
    br = _run_cores(np.asarray(x), np.asarray(y))
    return _combine(br.results)

